# revision 1
# baseline (speedup 1.0000x reference)
"""Trainium2 Bass kernel for nn_Attention_87454124081916 (sparse local-window attention).

Reference computation (per batch b, length n=4096, dim=512, 8 heads x 64):
  q = seq @ Wq + bq ; k,v = split(seq @ Wkv) ; g = sigmoid(seq @ Wg + bg)
  local window attention (window=128, keys = prev/cur/next window) with additive
  bias band from attn_bias, softmax over the 384 keys
  out = (attn_out * g) @ Wout

Sharding: 8 cores = 2 batches x 4 sequence chunks of 1024 rows.  Each core gets
its q rows plus a 128-row k/v halo on each side (zero-padded at batch edges) and
the pre-sliced bias band for its rows (out-of-range keys filled with -1e30).
No cross-core communication.

Device dataflow (per core) avoids all transposes of softmax weights:
  - seq chunk is transposed once on the PE (dim on partitions) -> seqT
  - qT,kT computed in [inner, n] layout; v in natural [n, inner] layout
  - S is computed TRANSPOSED per key-tile j: S^T_j = K_j q^T, so exp needs no
    max-subtraction (logits are O(1)) and P^T feeds the PV matmul directly
    (contraction over keys on the partition axis).
  - bias band is PE-transposed once per core and exp'd into two per-half packed
    tiles: P^T = exp(S^T) * exp(bias^T), one wide multiply per (head, half)
  - PV appends a ones-column to V so the softmax denominator Z falls out of the
    same matmul; overlapping windows accumulate via three bank-aligned PSUM
    "classes" (j mod 3) summed afterwards.
  - softmax normalizer 1/Z = exp(-ln Z) and the gate
    sigmoid(y) = exp(-ln(1+e^{-y})) use only Exp/Ln (plus Copy), which share
    one ACT table set, so there are no ACT table reloads anywhere.
  - output projection consumes X^T = (O^T/Z)*sigmoid directly as lhsT; each
    q-half's output projection overlaps the other half's attention on the PE.
All matmuls run in float32r (full-rate fp32); the P^T/exp(bias)/V path is bf16.
"""

import os
import sys
from contextlib import ExitStack

import numpy as np

for _p in ("/opt/trn_rl_repo",):
    if _p not in sys.path:
        sys.path.insert(0, _p)

import concourse.bacc as bacc
import concourse.bass as bass
import concourse.hw_specs as hw_specs
import concourse.mybir as mybir
import concourse.tile as tile
from concourse.bass_utils import run_bass_kernel_spmd

F32 = mybir.dt.float32
F32R = mybir.dt.float32r
AF = mybir.ActivationFunctionType

P = 128          # partitions / window size
DIM = 512
INNER = 512
H = 8
D = 64
W = 128
NQ = 8           # q tiles per core
NKV = 10         # kv tiles per core (incl. 1-tile halo each side)
NQROWS = NQ * W      # 1024
NKVROWS = NKV * W    # 1280
NEG = -1.0e30
SCALE = float(D) ** -0.5

# float32r: full-rate fp32 matmul with reduced internal precision.  walrus
# requires matmul operands to be *written* as float32r, so tiles feeding
# matmuls are declared with this dtype and producers cast on write.
MMT = F32R
# bf16 for the attention-probability path (P^T, exp(bias^T), V): halves the
# DVE elementwise cost (2x packed mode) at ~0.4% relative error on P*V,
# far inside the correctness gate; matmul accumulation stays fp32 in PSUM
BF16 = mybir.dt.bfloat16

_DBG = os.environ.get("KDBG") == "1"


# This kernel's only transcendentals are Exp and Ln.  The ACT table-set picker
# takes the first set containing each function, which puts Exp in
# 'exp_and_others' and Ln in 'natural_log' and reloads the table RAM on every
# alternation (~1.3us each, ~50 reloads).  Steer both to the combined
# 'natural_log_exp_and_others' set by hiding Exp/Ln from the other sets in the
# table map handed to the placement pass (names and order are kept, so the
# emitted act_func_set_id indices stay aligned with act_info.json and the
# runtime tables genuinely contain the functions used).
_orig_get_activation_tables = hw_specs.get_activation_tables


def _combined_act_tables(arch):
    tabs = dict(_orig_get_activation_tables(arch))
    exp_f = mybir.ActivationFunctionType.Exp
    ln_f = mybir.ActivationFunctionType.Ln
    out = {}
    for name, funcs in tabs.items():
        if name != "natural_log_exp_and_others":
            funcs = {f for f in funcs if f not in (exp_f, ln_f)}
        out[name] = funcs
    return out


bacc.get_activation_tables = _combined_act_tables


def _mm(ap):
    # dram-side bitcast for DMA into float32r tiles (bit-identical copy)
    return ap.bitcast(MMT) if MMT is not F32 else ap


def _q_window(j):
    """local q-tile window (inclusive) served by local kv tile j."""
    return max(0, j - 2), min(NQ - 1, j)


def _q_window_half(j, half):
    lo, hi = _q_window(j)
    return max(lo, half * 4), min(hi, half * 4 + 3)


_HALF_JS = {0: [0, 1, 2, 3, 4, 5], 1: [4, 5, 6, 7, 8, 9]}
# sim/exp groups: js grouped so each group's widths sum to 3 tiles (=384 cols,
# one PSUM bank), letting one exp cover the group with no garbage reads
_SIM_GROUPS = {0: [[0, 1], [2], [3], [4, 5]], 1: [[4, 5], [6], [7], [8, 9]]}
# accumulation-group first/last j per (half, j%3)
_CLS_FIRST = {0: {0: 0, 1: 1, 2: 2}, 1: {0: 6, 1: 4, 2: 5}}
_CLS_LAST = {0: {0: 3, 1: 4, 2: 5}, 1: {0: 9, 1: 7, 2: 8}}

# packed column offsets (j order) shared by the P^T tile and the exp(bias^T)
# tiles; total width is 1536 per half
_OFFS = {}
for _half in (0, 1):
    _OFFS[_half] = {}
    _cum = 0
    for _j in _HALF_JS[_half]:
        _lo, _hi = _q_window_half(_j, _half)
        _OFFS[_half][_j] = _cum
        _cum += (_hi - _lo + 1) * W
assert _cum == 1536


def _build_program(nreps=1):
    nc = bacc.Bacc("TRN2", target_bir_lowering=False, debug=False)

    seq_kv = nc.dram_tensor("seq_kv", [NKVROWS, DIM], F32, kind="ExternalInput").ap()
    band_d = nc.dram_tensor("band", [NQ, W, 3 * W], F32, kind="ExternalInput").ap()
    Wq_d = nc.dram_tensor("Wq", [DIM, INNER], F32, kind="ExternalInput").ap()
    Wkv_d = nc.dram_tensor("Wkv", [DIM, 2 * INNER], F32, kind="ExternalInput").ap()
    Wg_d = nc.dram_tensor("Wg", [DIM, INNER], F32, kind="ExternalInput").ap()
    Wout_d = nc.dram_tensor("Wout", [INNER, DIM], F32, kind="ExternalInput").ap()
    bq_d = nc.dram_tensor("bq", [INNER], F32, kind="ExternalInput").ap()
    bg_d = nc.dram_tensor("bg", [INNER], F32, kind="ExternalInput").ap()
    out_d = nc.dram_tensor("out", [NQROWS, DIM], F32, kind="ExternalOutput").ap()

    dbg = {}
    if _DBG:
        for nm, shp in [("dbg_qT", [P, NQROWS]), ("dbg_kT", [P, NKVROWS]),
                        ("dbg_v", [P, H * (D + 1)]), ("dbg_eb", [P, 384]),
                        ("dbg_pt", [P, 1536]), ("dbg_otz", [D + 1, 512]),
                        ("dbg_zr", [1, 512]), ("dbg_rep", [D, 512]),
                        ("dbg_xt", [P, NQROWS]), ("dbg_gT", [P, NQROWS]),
                        ("dbg_seqT", [P, NKVROWS])]:
            dbg[nm] = nc.dram_tensor(nm, shp, F32, kind="ExternalOutput").ap()

    eye_d = nc.inline_tensor(np.eye(P, dtype=np.float32), name="eye").ap()

    with tile.TileContext(nc) as tc:
      for _rep in range(nreps):
        with ExitStack() as ctx:
            # ------------------------------------------------------------------
            # persistent pools
            # ------------------------------------------------------------------
            wpool = ctx.enter_context(tc.tile_pool(name="wpool", bufs=1))
            apool = ctx.enter_context(tc.tile_pool(name="apool", bufs=1))

            # ---- weights: one DMA per matrix, [128, ktile, n] layout; issue
            # order puts eye+seq first (transposes gate everything), then the
            # projection weights, so compute starts while later DMAs stream in
            eye = wpool.tile([P, P], MMT, name="eye_sb", tag="eye_sb")
            nc.sync.dma_start(eye[:], _mm(eye_d[:]))
            wq_a = wpool.tile([P, 4, INNER], MMT, name="wq_a", tag="wq_a")
            wk_a = wpool.tile([P, 4, INNER], MMT, name="wk_a", tag="wk_a")
            wv_a = wpool.tile([P, 4, INNER], MMT, name="wv_a", tag="wv_a")
            wg_a = wpool.tile([P, 4, INNER], MMT, name="wg_a", tag="wg_a")
            wo_a = wpool.tile([P, 4, DIM], MMT, name="wo_a", tag="wo_a")
            bqs = wpool.tile([P, 4], F32, name="bqs", tag="bqs")
            bgs = wpool.tile([P, 4], F32, name="bgs", tag="bgs")

            # ---- persistent activations --------------------------------------
            # transposed activations [128, ktile, n]; head h lives at partition
            # rows (h%2)*64 of ktile h//2
            qT = apool.tile([P, 4, NQROWS], MMT, name="qT", tag="qT")
            kT = apool.tile([P, 4, NKVROWS], MMT, name="kT", tag="kT")
            gT = apool.tile([P, 4, NQROWS], F32, name="gT", tag="gT")
            xT = apool.tile([P, 4, NQROWS], MMT, name="xT", tag="xT")
            # v natural, padded per head with a ones column: [128, 10, 8, 65]
            vpa = apool.tile([P, NKV, H, D + 1], BF16, name="vpa", tag="vpa")
            # exp(bias^T) packed per half: [128 keys, 1536]
            ebh = [apool.tile([P, 1536], BF16, name=f"ebh{i}", tag=f"ebh{i}")
                   for i in (0, 1)]

            # ------------------------------------------------------------------
            # stages A+B: bias band and projections, overlapped (disjoint PSUM)
            # ------------------------------------------------------------------
            with tc.tile_pool(name="bandp", bufs=1) as bandp, \
                 tc.tile_pool(name="seqtp", bufs=1) as seqtp, \
                 tc.tile_pool(name="btp", bufs=2, space="PSUM") as btp, \
                 tc.tile_pool(name="trps", bufs=2, space="PSUM") as trps, \
                 tc.tile_pool(name="pjps", bufs=4, space="PSUM") as pjps:
                # ---- seq load + transpose -> seqT ----------------------------
                seqT = seqtp.tile([P, 4, NKVROWS], MMT, name="seqT", tag="seqT")
                seq_all = seqtp.tile([P, NKV, DIM], MMT, name="seq_all",
                                     tag="seq_all")
                seq_re = _mm(seq_kv.rearrange("(a p) n -> p a n", p=P))
                nc.sync.dma_start(seq_all[:, 0:2, :], seq_re[:, 0:2, :])
                band_all = bandp.tile([P, NQ, 3 * W], MMT, name="band_all",
                                      tag="band_all")
                nc.sync.dma_start(band_all[:],
                                  _mm(band_d.rearrange("i p w -> p i w")))
                nc.sync.dma_start(seq_all[:, 2:5, :], seq_re[:, 2:5, :])
                nc.sync.dma_start(seq_all[:, 5:8, :], seq_re[:, 5:8, :])
                nc.sync.dma_start(seq_all[:, 8:10, :], seq_re[:, 8:10, :])
                nc.sync.dma_start(
                    wv_a[:],
                    _mm(Wkv_d[:, INNER:2 * INNER].rearrange("(a p) n -> p a n", p=P)))
                nc.sync.dma_start(wq_a[:], _mm(Wq_d.rearrange("(a p) n -> p a n", p=P)))
                nc.sync.dma_start(
                    wk_a[:], _mm(Wkv_d[:, 0:INNER].rearrange("(a p) n -> p a n", p=P)))
                nc.sync.dma_start(bqs[:], bq_d.rearrange("(m p) -> p m", p=P))
                nc.sync.dma_start(bgs[:], bg_d.rearrange("(m p) -> p m", p=P))
                # attention scale folded into bq; bg negated for Exp(-(y+bg))
                nc.vector.tensor_scalar_mul(bqs[:], bqs[:], SCALE)
                nc.vector.tensor_scalar_mul(bgs[:], bgs[:], -1.0)
                nc.sync.dma_start(wg_a[:], _mm(Wg_d.rearrange("(a p) n -> p a n", p=P)))
                nc.sync.dma_start(wo_a[:], _mm(Wout_d.rearrange("(a p) n -> p a n", p=P)))
                # ones column written via ACT Copy(0*x+1)
                nc.scalar.activation(
                    vpa[:, :, :, D:D + 1],
                    eye[:, 0:NKV * H].rearrange("p (a b c) -> p a b c",
                                                b=H, c=1),
                    AF.Copy, bias=1.0, scale=0.0,
                )
                # (band DMA issued above, right after seq chunk 1)
                for j in range(NKV):
                    glo, ghi = _q_window(j)
                    bt = btp.tile([P, 384], MMT, name="bt", tag="bt")
                    for i in range(glo, ghi + 1):
                        c = j - i  # which 128-block of band tile i holds key tile j
                        blk = i - glo
                        nc.tensor.transpose(
                            bt[:, blk * W:(blk + 1) * W],
                            band_all[:, i, c * W:(c + 1) * W],
                            eye[:],
                        )
                    for half in (0, 1):
                        if j not in _HALF_JS[half]:
                            continue
                        lo, hi = _q_window_half(j, half)
                        nc.scalar.activation(
                            ebh[half][:, _OFFS[half][j]:
                                      _OFFS[half][j] + (hi - lo + 1) * W],
                            bt[:, (lo - glo) * W:(hi + 1 - glo) * W], AF.Exp)

                # transpose each seq tile, then immediately project its v row
                # block so attention's PV inputs are ready as early as possible
                for nt in range(NKV):
                    tp = trps.tile([P, 512], MMT, name="trt", tag="trt")
                    for kk in range(4):
                        nc.tensor.transpose(
                            tp[:, kk * P:(kk + 1) * P],
                            seq_all[:, nt, kk * P:(kk + 1) * P], eye[:])
                    nc.vector.tensor_copy(
                        seqT[:, :, nt * P:(nt + 1) * P],
                        tp[:].rearrange("p (a c) -> p a c", c=P))
                    pv_ = pjps.tile([P, 512], F32, name="pv_", tag="pj")
                    for kk in range(4):
                        nc.tensor.matmul(
                            pv_[:],
                            seqT[:, kk, nt * P:(nt + 1) * P],
                            wv_a[:, kk, :],
                            start=(kk == 0), stop=(kk == 3),
                        )
                    nc.vector.tensor_copy(
                        vpa[:, nt, :, 0:D],
                        pv_[:].rearrange("p (h e) -> p h e", e=D),
                    )

                # q / k / gate projections, per head-pair m (attention for
                # head pair m can start as soon as its slices land)
                for m in range(4):
                    for s2 in range(2):
                        cols = slice(W + s2 * 512, W + (s2 + 1) * 512)
                        pq = pjps.tile([P, 512], F32, name="pq", tag="pj")
                        for kk in range(4):
                            nc.tensor.matmul(
                                pq[:],
                                wq_a[:, kk, m * P:(m + 1) * P],
                                seqT[:, kk, cols],
                                start=(kk == 0), stop=(kk == 3),
                            )
                        nc.vector.tensor_scalar(
                            qT[:, m, s2 * 512:(s2 + 1) * 512], pq[:],
                            SCALE, bqs[:, m:m + 1],
                            mybir.AluOpType.mult, mybir.AluOpType.add,
                        )
                    for s3 in range(3):
                        wdt = 512 if s3 < 2 else 256
                        cols = slice(s3 * 512, s3 * 512 + wdt)
                        pk = pjps.tile([P, 512], F32, name="pk", tag="pj")
                        for kk in range(4):
                            nc.tensor.matmul(
                                pk[:, 0:wdt],
                                wk_a[:, kk, m * P:(m + 1) * P],
                                seqT[:, kk, cols],
                                start=(kk == 0), stop=(kk == 3),
                            )
                        nc.vector.tensor_copy(kT[:, m, cols], pk[:, 0:wdt])
                    for s2 in range(2):
                        cols = slice(W + s2 * 512, W + (s2 + 1) * 512)
                        pg = pjps.tile([P, 512], F32, name="pg", tag="pj")
                        for kk in range(4):
                            nc.tensor.matmul(
                                pg[:],
                                wg_a[:, kk, m * P:(m + 1) * P],
                                seqT[:, kk, cols],
                                start=(kk == 0), stop=(kk == 3),
                            )
                        # gate via exp/ln only (one ACT table set):
                        # gT := exp(-ln(1+e^{-(y+bg)})) = sigmoid(y+bg)
                        gs = gT[:, m, s2 * 512:(s2 + 1) * 512]
                        nc.scalar.activation(gs, pg[:], AF.Exp,
                                             bias=bgs[:, m:m + 1], scale=-1.0)
                        nc.scalar.activation(gs, gs, AF.Ln, bias=1.0)
                        nc.scalar.activation(gs, gs, AF.Exp, scale=-1.0)
                if _DBG:
                    nc.sync.dma_start(dbg["dbg_seqT"], seqT[:, 0, :].bitcast(F32))

            # ------------------------------------------------------------------
            # stage C: attention, processed per (q-half, head)
            # ------------------------------------------------------------------
            with tc.tile_pool(name="ptp", bufs=5) as ptp, \
                 tc.tile_pool(name="otzp", bufs=3) as otzp, \
                 tc.tile_pool(name="zrp", bufs=3) as zrp, \
                 tc.tile_pool(name="repp", bufs=3) as repp, \
                 tc.tile_pool(name="osb", bufs=4) as osb, \
                 tc.tile_pool(name="stp", bufs=1, space="PSUM") as stp, \
                 tc.tile_pool(name="clsp", bufs=3, space="PSUM") as clsp, \
                 tc.tile_pool(name="ops", bufs=3, space="PSUM") as ops:
                for half in (0, 1):
                    for h in range(H):
                        m, r0 = h // 2, (h % 2) * D
                        # ---- S^T, exp -> P^T, * exp(bias^T) --------------------
                        pt = ptp.tile([P, 1536], BF16, name="pt", tag="pt")
                        gpairs = [_SIM_GROUPS[half][0:2], _SIM_GROUPS[half][2:4]]
                        for pi, pair in enumerate(gpairs):
                            st = stp.tile([P, 1024], F32, name="st", tag="st")
                            base = _OFFS[half][pair[0][0]]
                            for gi, grp in enumerate(pair):
                                off = gi * 512
                                for j in grp:
                                    lo, hi = _q_window_half(j, half)
                                    wdt = (hi - lo + 1) * W
                                    nc.tensor.matmul(
                                        st[:, off:off + wdt],
                                        kT[r0:r0 + D, m, j * W:(j + 1) * W],
                                        qT[r0:r0 + D, m, lo * W:(hi + 1) * W],
                                        start=True, stop=True,
                                    )
                                    off += wdt
                            nc.scalar.activation(
                                pt[:, base:base + 768].rearrange(
                                    "p (a c) -> p a c", c=384),
                                st[:].rearrange("p (a c) -> p a c",
                                                c=512)[:, :, 0:384],
                                AF.Exp)
                        nc.vector.tensor_mul(pt[:, 0:768], pt[:, 0:768],
                                             ebh[half][:, 0:768])
                        nc.vector.tensor_mul(pt[:, 768:1536], pt[:, 768:1536],
                                             ebh[half][:, 768:1536])
                        # ---- PV (+ ones row -> Z): per-q-tile accumulation
                        # directly in one PSUM bank (bf16 P^T runs full-rate at
                        # N=128, and each q-tile's 3-j group start/stops before
                        # the next begins, so one bank suffices) ---------------
                        otz = clsp.tile([D + 1, 512], F32, name="otz", tag="otz")
                        for qi in range(4):
                            gq = half * 4 + qi  # core-local q tile
                            js = [j for j in _HALF_JS[half]
                                  if _q_window_half(j, half)[0] <= gq
                                  <= _q_window_half(j, half)[1]]
                            for ji, j in enumerate(js):
                                lo = _q_window_half(j, half)[0]
                                off = _OFFS[half][j] + (gq - lo) * W
                                nc.tensor.matmul(
                                    otz[:, qi * W:(qi + 1) * W],
                                    vpa[:, j, h, :],
                                    pt[:, off:off + W],
                                    start=(ji == 0), stop=(ji == len(js) - 1),
                                )
                        # ---- normalize + gate ----------------------------------
                        # X = O * (1/Z) * sigmoid(y) with 1/Z = exp(-ln Z)
                        # (stock Ln/Exp only: the custom-DVE reciprocal ops
                        # produce garbage under this runtime, and Sigmoid would
                        # force ACT table reloads)
                        zln = zrp.tile([1, 512], F32, name="zln", tag="zln")
                        nc.scalar.activation(zln[:], otz[D:D + 1, :], AF.Ln)
                        rep = repp.tile([D, 512], F32, name="rep", tag="rep")
                        nc.gpsimd.partition_broadcast(rep[:], zln[:])
                        nc.scalar.activation(rep[:], rep[:], AF.Exp, scale=-1.0)
                        xs = xT[r0:r0 + D, m, half * 512:(half + 1) * 512]
                        nc.vector.tensor_mul(xs, otz[0:D, :], rep[:])
                        nc.vector.tensor_mul(xs, xs,
                                             gT[r0:r0 + D, m,
                                                half * 512:(half + 1) * 512])
                        if _DBG and half == 0 and h == 0:
                            nc.sync.dma_start(dbg["dbg_pt"], pt[:].bitcast(F32))
                            nc.sync.dma_start(dbg["dbg_otz"], otz[:])
                            nc.sync.dma_start(dbg["dbg_zr"], zln[:])
                            nc.sync.dma_start(dbg["dbg_rep"], rep[:])
                    # ---- output projection for this half's q tiles: overlaps
                    # with the other half's attention on the PE ---------------
                    for t in range(half * 4, half * 4 + 4):
                        po = ops.tile([P, DIM], F32, name="po", tag="po")
                        for mm in range(4):
                            nc.tensor.matmul(
                                po[:],
                                xT[:, mm, t * P:(t + 1) * P],
                                wo_a[:, mm, :],
                                start=(mm == 0), stop=(mm == 3),
                            )
                        ot = osb.tile([P, DIM], F32, name="ot", tag="ot")
                        nc.vector.tensor_copy(ot[:], po[:])
                        nc.sync.dma_start(out_d[t * P:(t + 1) * P, :], ot[:])

            if _DBG:
                nc.sync.dma_start(dbg["dbg_qT"], qT[:, 0, :].bitcast(F32))
                nc.sync.dma_start(dbg["dbg_kT"], kT[:, 0, :].bitcast(F32))
                nc.sync.dma_start(dbg["dbg_v"],
                                  vpa[:, 4].rearrange("p h e -> p (h e)").bitcast(F32))
                nc.sync.dma_start(dbg["dbg_eb"], ebh[0][:, 0:384].bitcast(F32))
                nc.sync.dma_start(dbg["dbg_gT"], gT[:, 0, :])
                nc.sync.dma_start(dbg["dbg_xt"], xT[:, 0, :].bitcast(F32))


    nc.compile()
    return nc


_NC = {}
LAST_RESULT = None


def _get_nc(nreps=1):
    if nreps not in _NC:
        _NC[nreps] = _build_program(nreps)
    return _NC[nreps]


def _prep_inputs(seq, attn_bias, Wq, bq, Wkv, Wout, Wg, bg, mask):
    seq = np.ascontiguousarray(np.asarray(seq, dtype=np.float32))
    attn_bias = np.asarray(attn_bias, dtype=np.float32)
    Wq = np.ascontiguousarray(np.asarray(Wq, dtype=np.float32))
    Wkv = np.ascontiguousarray(np.asarray(Wkv, dtype=np.float32))
    Wout = np.ascontiguousarray(np.asarray(Wout, dtype=np.float32))
    Wg = np.ascontiguousarray(np.asarray(Wg, dtype=np.float32))
    bq = np.ascontiguousarray(np.asarray(bq, dtype=np.float32))
    bg = np.ascontiguousarray(np.asarray(bg, dtype=np.float32))
    b, n, dim = seq.shape
    SC = 4
    CH = n // SC
    in_maps = []
    for c in range(8):
        bi, sc = divmod(c, SC)
        r0 = sc * CH
        kv = np.zeros((NKVROWS, DIM), np.float32)
        lo, hi = r0 - W, r0 + CH + W
        slo, shi = max(lo, 0), min(hi, n)
        kv[slo - lo:shi - lo] = seq[bi, slo:shi]
        band = np.full((NQ, W, 3 * W), NEG, np.float32)
        for i in range(NQ):
            g = sc * NQ + i
            klo, khi = (g - 1) * W, (g + 2) * W
            sk_lo, sk_hi = max(klo, 0), min(khi, n)
            band[i, :, sk_lo - klo:sk_hi - klo] = \
                attn_bias[bi, g * W:(g + 1) * W, sk_lo:sk_hi]
        in_maps.append(dict(seq_kv=kv, band=band, Wq=Wq, Wkv=Wkv, Wg=Wg,
                            Wout=Wout, bq=bq, bg=bg))
    return in_maps


def kernel(seq, attn_bias, Wq, bq, Wkv, Wout, Wg, bg, mask):
    global LAST_RESULT
    nc = _get_nc()
    in_maps = _prep_inputs(seq, attn_bias, Wq, bq, Wkv, Wout, Wg, bg, mask)
    res = run_bass_kernel_spmd(nc, in_maps, core_ids=list(range(8)))
    LAST_RESULT = res
    b, n, dim = np.asarray(seq).shape
    out = np.empty((b, n, dim), np.float32)
    for c in range(8):
        bi, sc = divmod(c, 4)
        out[bi, sc * NQROWS:(sc + 1) * NQROWS] = res.results[c]["out"]
    return out


if __name__ == "__main__":
    rng = np.random.default_rng(0)
    seq = rng.standard_normal((2, 4096, 512), dtype=np.float32)
    bias = rng.standard_normal((2, 4096, 4096), dtype=np.float32) * 0.1
    Wq = rng.standard_normal((512, 512), dtype=np.float32) * 0.02
    Wkv = rng.standard_normal((512, 1024), dtype=np.float32) * 0.02
    Wout = rng.standard_normal((512, 512), dtype=np.float32) * 0.02
    Wg = rng.standard_normal((512, 512), dtype=np.float32) * 0.02
    bq = np.zeros(512, np.float32)
    bg = np.ones(512, np.float32)
    mask = np.ones((2, 4096), bool)
    out = kernel(seq, bias, Wq, bq, Wkv, Wout, Wg, bg, mask)
    print(out.shape, out.dtype)



# revision 59
# speedup vs baseline: 1.5098x; 1.5098x over previous
"""Trainium2 Bass kernel for nn_Attention_87454124081916 (sparse local-window attention).

Reference computation (per batch b, length n=4096, dim=512, 8 heads x 64):
  q = seq @ Wq + bq ; k,v = split(seq @ Wkv) ; g = sigmoid(seq @ Wg + bg)
  local window attention (window=128, keys = prev/cur/next window) with additive
  bias band from attn_bias, softmax over the 384 keys
  out = (attn_out * g) @ Wout

Sharding: 8 cores = 2 batches x 4 sequence chunks of 1024 rows.  Each core gets
its q rows plus a 128-row k/v halo on each side (zero-padded at batch edges) and
the pre-sliced bias band for its rows (out-of-range keys filled with -1e30).
No cross-core communication.  Host prep is layout-only plus a bf16 downcast of
seq/weights/band for the wire (halves the serial input-DMA stream; all matmul
accumulation stays fp32 in PSUM): seq arrives pre-transposed ([dim, rows]) and
the bias band arrives transposed and packed per q-half ([keys, q-slots]), so
the device does no PE transposes of inputs and exps the band in one ACT op per
half.

Device dataflow (per core), everything bf16 on the PE at 1 cyc/row:
  - qT,kT computed in [inner, n] layout; v in natural [n, inner] layout with a
    ones column appended per head (so the softmax denominator Z falls out of
    the PV matmul).
  - S is computed TRANSPOSED per key-tile j: S^T_j = K_j q^T with the
    attention scale folded into the exp's scale operand; exp needs no
    max-subtraction (logits are O(1)).  Both halves' S/exp streams run during
    the projection phase, right behind each head-pair's q/k projections, so
    the ACT exp stream (the second-longest engine stream) starts as early as
    the DMA allows and runs unbroken.
  - P^T = exp(scale*S^T) * exp(bias^T) in bf16; half-1's multiplies run on the
    otherwise idle gpsimd engine.
  - PV runs in NATURAL orientation: O[q,65] = P^T(lhsT) @ V, free dim 65 --
    half the PE columns of the transposed form -- and puts the per-q softmax
    normalizer on the PARTITION axis where it is cheap: rz = 1/Z on [128,8]
    per q-tile via a bit-trick+Newton reciprocal on DVE (keeps the epilogue
    off the ACT table ops), applied via a stride-0-broadcast DVE multiply.
  - gates are computed in natural layout with bg added via a K=1
    outer-product matmul and a one-pass table Sigmoid; the gate block is
    contiguous in the ACT stream so it costs exactly two table loads, and its
    PE matmuls fill the epilogue while exps drain.
  - X = O * g * rz in bf16 is PE-transposed per 128-block into the lhsT of
    the output projection.
  - PSUM is re-scoped mid-program: the S pools close after the last exp so
    the epilogue runs with a double-buffered otz pool and a 3-deep shared
    [P,512] pool for gates / late v-tiles / out-proj.
"""


import os
import sys
from contextlib import ExitStack

import numpy as np

for _p in ("/opt/trn_rl_repo",):
    if _p not in sys.path:
        sys.path.insert(0, _p)

import concourse.bacc as bacc
import concourse.bass as bass
import concourse.hw_specs as hw_specs
import concourse.mybir as mybir
import concourse.tile as tile
from concourse.bass_utils import run_bass_kernel_spmd

F32 = mybir.dt.float32
F32R = mybir.dt.float32r
AF = mybir.ActivationFunctionType

P = 128          # partitions / window size
DIM = 512
INNER = 512
H = 8
D = 64
W = 128
NQ = 8           # q tiles per core
NKV = 10         # kv tiles per core (incl. 1-tile halo each side)
NQROWS = NQ * W      # 1024
NKVROWS = NKV * W    # 1280
NEG = -1.0e30
SCALE = float(D) ** -0.5

# float32r: full-rate fp32 matmul with reduced internal precision.  walrus
# requires matmul operands to be *written* as float32r, so tiles feeding
# matmuls are declared with this dtype and producers cast on write.
MMT = F32R
# bf16 for the attention-probability path (P^T, exp(bias^T), V, X): halves the
# DVE elementwise cost (2x/4x packed modes) at ~0.4% relative error, far
# inside the correctness gate; matmul accumulation stays fp32 in PSUM.
BF16 = mybir.dt.bfloat16

_DBG = os.environ.get("KDBG") == "1"
# ship seq + projection weights to the device in bf16: halves the serial
# input-DMA stream (the projection matmuls then run bf16/bf16; PSUM
# accumulation stays fp32)
WIRE_BF16 = os.environ.get("KF32_WIRE") != "1"


# This kernel's only transcendentals are Exp and Ln.  The ACT table-set picker
# takes the first set containing each function, which puts Exp in
# 'exp_and_others' and Ln in 'natural_log' and reloads the table RAM on every
# alternation (~1.3us each).  Steer both to the combined
# 'natural_log_exp_and_others' set by hiding Exp/Ln from the other sets in the
# table map handed to the placement pass (names and order are kept, so the
# emitted act_func_set_id indices stay aligned with act_info.json and the
# runtime tables genuinely contain the functions used).
_orig_get_activation_tables = hw_specs.get_activation_tables


def _combined_act_tables(arch):
    tabs = dict(_orig_get_activation_tables(arch))
    sig = mybir.ActivationFunctionType.Sigmoid
    out = {}
    for name, funcs in tabs.items():
        # the combined exp/ln set keeps everything; 'sigmoid_and_others'
        # keeps ONLY sigmoid (so Copy/Identity still resolve to the combined
        # set); every other set is emptied.  Result: one sigmoid table trip
        # for the gate block (2 loads), zero other reloads.
        if name == "natural_log_exp_and_others":
            out[name] = funcs
        elif name == "sigmoid_and_others":
            out[name] = {f for f in funcs if f == sig}
        else:
            out[name] = set()
    return out


bacc.get_activation_tables = _combined_act_tables


def _mm(ap):
    # dram-side bitcast for DMA into float32r tiles (bit-identical copy)
    return ap.bitcast(MMT) if ap.dtype == F32 and MMT is not F32 else ap


def _q_window(j):
    """local q-tile window (inclusive) served by local kv tile j."""
    return max(0, j - 2), min(NQ - 1, j)


def _q_window_half(j, half):
    lo, hi = _q_window(j)
    return max(lo, half * 4), min(hi, half * 4 + 3)


_HALF_JS = {0: [0, 1, 2, 3, 4, 5], 1: [4, 5, 6, 7, 8, 9]}

# packed column offsets (j order) shared by the P^T tile and the exp(bias^T)
# tiles; total width is 1536 per half
_OFFS = {}
for _half in (0, 1):
    _OFFS[_half] = {}
    _cum = 0
    for _j in _HALF_JS[_half]:
        _lo, _hi = _q_window_half(_j, _half)
        _OFFS[_half][_j] = _cum
        _cum += (_hi - _lo + 1) * W
    assert _cum == 1536

# S^T PSUM layout: per (half, head) two [P,1024] tiles; each tile holds two
# 384-wide groups at bank offsets 0 and 512 (matmul writes may not cross the
# 512-col PSUM bank boundary).  Each entry is (tile_idx, [(j, dst_off, qlo,
# qhi), ...]): the matmul for j writes q tiles qlo..qhi (padded to >=2 tiles
# so the fp32r free dim is >=256 and the PE runs 1 cyc/row) at tile cols
# dst_off...; emission order guarantees garbage columns from the padding are
# overwritten by later real writes.
# bf16 S operands run 1 cyc/row at any free width, so no fp32r >=256 padding
_S_LAYOUT = {
    0: [(0, [(0, 0, 0, 0), (1, 128, 0, 1), (2, 512, 0, 2)]),
        (1, [(3, 0, 1, 3), (4, 512, 2, 3), (5, 768, 3, 3)])],
    1: [(0, [(4, 0, 4, 4), (5, 128, 4, 5), (6, 512, 4, 6)]),
        (1, [(7, 0, 5, 7), (8, 512, 6, 7), (9, 768, 7, 7)])],
}


def _build_program(nreps=1):
    nc = bacc.Bacc("TRN2", target_bir_lowering=False, debug=False)

    WT = BF16 if WIRE_BF16 else F32
    seqT_d = nc.dram_tensor("seqT", [DIM, NKVROWS], WT, kind="ExternalInput").ap()
    bandT_d = nc.dram_tensor("bandT", [2, P, 1536], WT, kind="ExternalInput").ap()
    Wq_d = nc.dram_tensor("Wq", [DIM, INNER], WT, kind="ExternalInput").ap()
    Wkv_d = nc.dram_tensor("Wkv", [DIM, 2 * INNER], WT, kind="ExternalInput").ap()
    Wg_d = nc.dram_tensor("Wg", [DIM, INNER], WT, kind="ExternalInput").ap()
    Wout_d = nc.dram_tensor("Wout", [INNER, DIM], WT, kind="ExternalInput").ap()
    bq_d = nc.dram_tensor("bq", [INNER], F32, kind="ExternalInput").ap()
    bg_d = nc.dram_tensor("bg", [INNER], WT, kind="ExternalInput").ap()
    out_d = nc.dram_tensor("out", [NQROWS, DIM], F32, kind="ExternalOutput").ap()

    dbg = {}
    if _DBG:
        for nm, shp in [("dbg_qT", [P, NQROWS]), ("dbg_kT", [P, NKVROWS]),
                        ("dbg_v", [P, H * (D + 1)]), ("dbg_eb", [P, 1536]),
                        ("dbg_pt", [P, 1536]), ("dbg_otz", [P, 1024]),
                        ("dbg_rz", [P, 8]), ("dbg_gn", [P, 512]),
                        ("dbg_xt", [P, NQROWS]), ("dbg_seqT", [P, NKVROWS])]:
            dbg[nm] = nc.dram_tensor(nm, shp, F32, kind="ExternalOutput").ap()

    import ml_dtypes
    eye_d = nc.inline_tensor(np.eye(P).astype(ml_dtypes.bfloat16),
                             name="eye").ap()
    ones_d = nc.inline_tensor(
        np.ones((1, P)).astype(ml_dtypes.bfloat16 if WIRE_BF16
                               else np.float32), name="ones").ap()

    with tile.TileContext(nc) as tc:
      for _rep in range(nreps):
        with ExitStack() as ctx:
            # ------------------------------------------------------------------
            # persistent pools + DMA wave
            # ------------------------------------------------------------------
            wpool = ctx.enter_context(tc.tile_pool(name="wpool", bufs=1))
            apool = ctx.enter_context(tc.tile_pool(name="apool", bufs=1))

            WMT = BF16 if WIRE_BF16 else MMT
            seqT = apool.tile([P, 4, NKVROWS], WMT, name="seqT", tag="seqT")
            seq_re = _mm(seqT_d.rearrange("(a p) n -> p a n", p=P))
            bqs = wpool.tile([P, 4], F32, name="bqs", tag="bqs")
            WMTX = BF16 if WIRE_BF16 else MMT
            bgr = wpool.tile([1, INNER], WMTX, name="bgr", tag="bgr")
            ones1 = wpool.tile([1, P], WMTX, name="ones1", tag="ones1")
            # seq chunks stream on the gpsimd software-DGE queue, weights on
            # the SP hardware queue: two parallel input streams instead of
            # one serial 28us stream.
            nc.sync.dma_start(bgr[:], _mm(bg_d.rearrange("(a n) -> a n", a=1)))
            eye_bf = wpool.tile([P, P], BF16, name="eye_bf_sb", tag="eye_bf")
            nc.sync.dma_start(eye_bf[:], eye_d[:])
            nc.sync.dma_start(ones1[:], _mm(ones_d[:]))
            nc.sync.dma_start(bqs[:], bq_d.rearrange("(m p) -> p m", p=P))
            wq_a = wpool.tile([P, 4, INNER], WMT, name="wq_a", tag="wq_a")
            wk_a = wpool.tile([P, 4, INNER], WMT, name="wk_a", tag="wk_a")
            wq_re = _mm(Wq_d.rearrange("(a p) n -> p a n", p=P))
            wk_re = _mm(Wkv_d[:, 0:INNER].rearrange("(a p) n -> p a n", p=P))
            nc.sync.dma_start(seqT[:, :, 0:256], seq_re[:, :, 0:256])
            nc.sync.dma_start(seqT[:, :, 256:768], seq_re[:, :, 256:768])
            cols = slice(0, P)
            nc.sync.dma_start(wq_a[:, :, cols], wq_re[:, :, cols])
            nc.sync.dma_start(wk_a[:, :, cols], wk_re[:, :, cols])
            nc.sync.dma_start(seqT[:, :, 768:1280], seq_re[:, :, 768:1280])
            bsbs = []
            for half in (0, 1):
                bsb = wpool.tile([P, 1536], BF16 if WIRE_BF16 else F32,
                                 name=f"bsb{half}", tag=f"bsb{half}")
                nc.sync.dma_start(bsb[:], bandT_d[half])
                bsbs.append(bsb)
            for m in range(1, 4):
                cols = slice(m * P, (m + 1) * P)
                nc.sync.dma_start(wq_a[:, :, cols], wq_re[:, :, cols])
                nc.sync.dma_start(wk_a[:, :, cols], wk_re[:, :, cols])
            wv_a = wpool.tile([P, 4, INNER], WMT, name="wv_a", tag="wv_a")
            nc.sync.dma_start(
                wv_a[:],
                _mm(Wkv_d[:, INNER:2 * INNER].rearrange("(a p) n -> p a n", p=P)))
            wg_a = wpool.tile([P, 4, INNER], WMT, name="wg_a", tag="wg_a")
            nc.sync.dma_start(wg_a[:], _mm(Wg_d.rearrange("(a p) n -> p a n", p=P)))
            wo_a = wpool.tile([P, 4, DIM], WMT, name="wo_a", tag="wo_a")
            nc.sync.dma_start(
                wo_a[:], _mm(Wout_d.rearrange("(a p) n -> p a n", p=P)))

            # ---- persistent activations --------------------------------------
            # transposed activations [128, ktile, n]; head h lives at partition
            # rows (h%2)*64 of ktile h//2
            qT = apool.tile([P, 4, NQROWS], BF16, name="qT", tag="qT")
            kT = apool.tile([P, 4, NKVROWS], BF16, name="kT", tag="kT")
            gn = apool.tile([P, NQ, INNER], BF16, name="gn", tag="gn")
            xTs = apool.tile([P, 4, NQROWS], BF16, name="xTs", tag="xTs")
            # v natural, padded per head with a ones column: [128, 10, 8, 65]
            vpa = apool.tile([P, NKV, H, D + 1], BF16, name="vpa", tag="vpa")
            # exp(bias^T) packed per half: [128 keys, 1536]
            ebh = [apool.tile([P, 1536], BF16, name=f"ebh{i}", tag=f"ebh{i}")
                   for i in (0, 1)]
            pts = {}   # (half, h) -> pt tile
            stp_cm = tc.tile_pool(name="stp", bufs=2, space="PSUM")
            stp = stp_cm.__enter__()

            def emit_s_head(half, h, ptp):
                """S^T matmuls + exp + exp(bias) multiply for one head."""
                m, r0 = h // 2, (h % 2) * D
                pt = ptp.tile([P, 1536], BF16, name="pt", tag="pt")
                pts[(half, h)] = pt
                for ti, writes in _S_LAYOUT[half]:
                    st = stp.tile([P, 1024], F32, name="st", tag="st")
                    for (j, dst, qlo, qhi) in writes:
                        nc.tensor.matmul(
                            st[:, dst:dst + (qhi - qlo + 1) * W],
                            kT[r0:r0 + D, m, j * W:(j + 1) * W],
                            qT[r0:r0 + D, m, qlo * W:(qhi + 1) * W],
                            start=True, stop=True,
                        )
                    nc.scalar.activation(
                        pt[:, ti * 768:(ti + 1) * 768].rearrange(
                            "p (a c) -> p a c", c=384),
                        st[:].rearrange("p (a c) -> p a c", c=512)[:, :, 0:384],
                        AF.Exp, scale=SCALE)
                # half-1 multiplies on the (otherwise idle) Pool engine:
                # slow but off the DVE spine, done before PV(1,*) needs them
                eng = nc.gpsimd if half == 1 else nc.vector
                eng.tensor_mul(pt[:], pt[:], ebh[half][:])

            # ------------------------------------------------------------------
            # phase 1: projections (+ gates on ACT's idle window) with the
            # half-0 S/exp stream interleaved per head-pair m
            # ------------------------------------------------------------------
            with tc.tile_pool(name="ptp", bufs=16) as ptp:
              with tc.tile_pool(name="bsbp", bufs=2) as bsbp, \
                   tc.tile_pool(name="wsp", bufs=1) as wsp, \
                   tc.tile_pool(name="pjps", bufs=4, space="PSUM") as pjps:

                def emit_gproj(t, pool, tag):
                    # natural-layout gate projection; bg added via a K=1
                    # outer-product matmul; one-pass sigmoid (own table set)
                    pg = pool.tile([P, 512], F32, name="pg", tag=tag)
                    nc.tensor.matmul(pg[:], ones1[:], bgr[:],
                                     start=True, stop=False)
                    for kk in range(4):
                        nc.tensor.matmul(
                            pg[:],
                            seqT[:, kk, (t + 1) * P:(t + 2) * P],
                            wg_a[:, kk, :],
                            start=False, stop=(kk == 3),
                        )
                    nc.scalar.activation(gn[:, t, :], pg[:], AF.Sigmoid)

                def emit_vproj(nt, pool, tag):
                    pv_ = pool.tile([P, 512], F32, name="pv_", tag=tag)
                    for kk in range(4):
                        nc.tensor.matmul(
                            pv_[:],
                            seqT[:, kk, nt * P:(nt + 1) * P],
                            wv_a[:, kk, :],
                            start=(kk == 0), stop=(kk == 3),
                        )
                    nc.vector.tensor_copy(
                        vpa[:, nt, :, 0:D],
                        pv_[:].rearrange("p (h e) -> p h e", e=D),
                    )

                # ones column via ACT Copy(0*x+1) -- walrus rejects non-zero
                # memsets for 2-byte dtypes
                nc.scalar.activation(
                    vpa[:, :, :, D:D + 1],
                    eye_bf[:, 0:NKV * H].rearrange("p (a b c) -> p a b c",
                                                   b=H, c=1),
                    AF.Copy, bias=1.0, scale=0.0)
                # band exp (band DMA'd early on SP)
                for half in (0, 1):
                    nc.scalar.activation(ebh[half][:], bsbs[half][:], AF.Exp)

                # q / k projections per head-pair m with BOTH halves' S/exp
                # streams right behind: the ACT exp stream starts as early
                # as possible and runs unbroken; v/g projections fill the PE
                # later, under the exps
                for m in range(4):
                    for s2 in range(2):
                        cols = slice(W + s2 * 512, W + (s2 + 1) * 512)
                        pq = pjps.tile([P, 512], F32, name="pq", tag="pj")
                        for kk in range(4):
                            nc.tensor.matmul(
                                pq[:],
                                wq_a[:, kk, m * P:(m + 1) * P],
                                seqT[:, kk, cols],
                                start=(kk == 0), stop=(kk == 3),
                            )
                        nc.vector.tensor_scalar_add(
                            qT[:, m, s2 * 512:(s2 + 1) * 512], pq[:],
                            bqs[:, m:m + 1])
                    for s3 in range(3):
                        wdt = 512 if s3 < 2 else 256
                        cols = slice(s3 * 512, s3 * 512 + wdt)
                        pk = pjps.tile([P, 512], F32, name="pk", tag="pj")
                        for kk in range(4):
                            nc.tensor.matmul(
                                pk[:, 0:wdt],
                                wk_a[:, kk, m * P:(m + 1) * P],
                                seqT[:, kk, cols],
                                start=(kk == 0), stop=(kk == 3),
                            )
                        nc.vector.tensor_copy(kT[:, m, cols], pk[:, 0:wdt])
                    emit_s_head(0, 2 * m, ptp)
                    emit_s_head(0, 2 * m + 1, ptp)
                    emit_s_head(1, 2 * m, ptp)
                    emit_s_head(1, 2 * m + 1, ptp)
                for nt in range(6):
                    emit_vproj(nt, pjps, "pj")

                if _DBG:
                    nc.sync.dma_start(dbg["dbg_seqT"], seqT[:, 0, :].bitcast(F32))



              # ----------------------------------------------------------------
              # phase 2: attention epilogue per q tile + half-1 S stream
              # (pjps closed: its PSUM banks are recycled for the epilogue)
              # ----------------------------------------------------------------
              if True:
                with tc.tile_pool(name="xgp", bufs=3) as xgp, \
                     tc.tile_pool(name="zrp", bufs=3) as zrp, \
                     tc.tile_pool(name="osb", bufs=3) as osb:

                    qstate = {}
                    pp = {}

                    def emit_pv(half, qi):
                        """PV matmuls + Z chain + gate multiply for one q
                        tile (the rz multiply is deferred to emit_xtr so the
                        ACT Ln/Exp latency stays off the DVE stream)."""
                        gq = half * 4 + qi
                        otz = pp["otz"].tile([P, 1024], F32, name="otz",
                                             tag="otz")
                        for h in range(H):
                            js = [j for j in _HALF_JS[half]
                                  if _q_window_half(j, half)[0] <= gq
                                  <= _q_window_half(j, half)[1]]
                            dst0 = (h // 4) * 512 + (h % 4) * (D + 1)
                            for ji, j in enumerate(js):
                                lo = _q_window_half(j, half)[0]
                                off = _OFFS[half][j] + (gq - lo) * W
                                nc.tensor.matmul(
                                    otz[:, dst0:dst0 + D + 1],
                                    pts[(half, h)][:, off:off + W],
                                    vpa[:, j, h, :],
                                    start=(ji == 0), stop=(ji == len(js) - 1),
                                )
                        # rz = 1/Z per (q row, head) [128,8] via a Newton
                        # step on DVE (bit-trick seed): keeps the epilogue
                        # entirely off the ACT table ops so the gate
                        # sigmoids stay contiguous (no table reloads)
                        zv = otz[:].rearrange("p (g s) -> p g s", g=2)[
                            :, :, 0:4 * (D + 1)].rearrange(
                            "p g (h e) -> p g h e", e=D + 1)
                        zcol = zv[:, :, :, D:D + 1]
                        I32 = mybir.dt.int32
                        zl = zrp.tile([P, 8], F32, name="zl", tag="zl")
                        nc.vector.tensor_scalar(
                            zl[:].bitcast(I32).rearrange(
                                "p (g h e) -> p g h e", g=2, e=1),
                            zcol.bitcast(I32), -1, 0x7EF311C3,
                            mybir.AluOpType.mult, mybir.AluOpType.add)
                        tt = zrp.tile([P, 8], F32, name="tt", tag="tt")
                        nc.vector.tensor_mul(
                            tt[:].rearrange("p (g h e) -> p g h e", g=2, e=1),
                            zcol, zl[:].rearrange("p (g h e) -> p g h e",
                                                  g=2, e=1))
                        nc.vector.tensor_scalar(
                            tt[:], tt[:], -1.0, 2.0,
                            mybir.AluOpType.mult, mybir.AluOpType.add)
                        rz = zrp.tile([P, 8], F32, name="rz", tag="rz")
                        nc.vector.tensor_mul(rz[:], zl[:], tt[:])
                        # gate multiply first -- it does not depend on rz
                        xg = xgp.tile([P, 512], BF16, name="xg", tag="xg")
                        nc.vector.tensor_mul(
                            xg[:].rearrange("p (g h e) -> p g h e", g=2, e=D),
                            zv[:, :, :, 0:D],
                            gn[:, gq, :].rearrange("p (g h e) -> p g h e",
                                                   g=2, e=D))
                        qstate[gq] = (xg, rz)
                        if _DBG and gq == 0:
                            nc.sync.dma_start(dbg["dbg_otz"], otz[:])
                            nc.sync.dma_start(dbg["dbg_rz"], rz[:])
                            nc.sync.dma_start(dbg["dbg_gn"], gn[:, 0, :])

                    def emit_xtr(half, qi):
                        """rz multiply + X transpose + xTs copy."""
                        gq = half * 4 + qi
                        xg, rz = qstate.pop(gq)
                        xn = xgp.tile([P, 512], BF16, name="xn", tag="xn")
                        # half-1: DVE is the tail bottleneck; Pool and ACT
                        # are idle there, so shift the rz multiply and the
                        # xTs copy off DVE
                        nc.vector.tensor_mul(
                            xn[:].rearrange("p (g h e) -> p g h e", g=2, e=D),
                            xg[:].rearrange("p (g h e) -> p g h e", g=2, e=D),
                            rz[:].rearrange("p (g h) -> p g h", g=2)
                                 .unsqueeze(3).broadcast_to([P, 2, 4, D]))
                        # transpose X (bf16, 1 cyc/row) into xTs
                        xt = pp["xt"].tile([P, 4, P], BF16, name="xt",
                                           tag="xt")
                        for b in range(4):
                            nc.tensor.transpose(
                                xt[:, b, :], xn[:, b * P:(b + 1) * P],
                                eye_bf[:])
                        nc.vector.tensor_copy(
                            xTs[:, :, gq * P:(gq + 1) * P], xt[:])

                    def emit_outproj(t):
                        po = pp["po"].tile([P, DIM], F32, name="po", tag="po")
                        for mm in range(4):
                            nc.tensor.matmul(
                                po[:],
                                xTs[:, mm, t * P:(t + 1) * P],
                                wo_a[:, mm, :],
                                start=(mm == 0), stop=(mm == 3),
                            )
                        ot = osb.tile([P, DIM], F32, name="ot", tag="ot")
                        if t >= 4:
                            nc.scalar.activation(ot[:], po[:], AF.Copy)
                        else:
                            nc.vector.tensor_copy(ot[:], po[:])
                        nc.sync.dma_start(out_d[t * P:(t + 1) * P, :], ot[:])

                    # ---- epilogues: S/exp all done in phase 1; stp is
                    # closed so the epilogue runs with double-buffered otz
                    # and a deep shared [P,512] pool for gates / late
                    # v-tiles / out-proj: otz(4) + xt(1) + po(3) = 8 banks.
                    stp_cm.__exit__(None, None, None)
                    with tc.tile_pool(name="otzp", bufs=2,
                                      space="PSUM") as otzp, \
                         tc.tile_pool(name="xtp", bufs=1,
                                      space="PSUM") as xtp, \
                         tc.tile_pool(name="pop", bufs=3,
                                      space="PSUM") as pop:
                        pp["otz"], pp["xt"], pp["po"] = otzp, xtp, pop
                        emit_gproj(0, pop, "po")
                        emit_gproj(1, pop, "po")
                        emit_pv(0, 0)
                        emit_gproj(2, pop, "po")
                        emit_pv(0, 1)
                        emit_xtr(0, 0)
                        emit_gproj(3, pop, "po")
                        emit_pv(0, 2)
                        emit_xtr(0, 1)
                        emit_outproj(0)
                        emit_gproj(4, pop, "po")
                        emit_pv(0, 3)
                        emit_xtr(0, 2)
                        emit_outproj(1)
                        emit_vproj(6, pop, "po")
                        emit_gproj(5, pop, "po")
                        emit_vproj(7, pop, "po")
                        emit_pv(1, 0)
                        emit_xtr(0, 3)
                        emit_outproj(2)
                        emit_gproj(6, pop, "po")
                        emit_vproj(8, pop, "po")
                        emit_pv(1, 1)
                        emit_xtr(1, 0)
                        emit_outproj(3)
                        emit_gproj(7, pop, "po")
                        emit_vproj(9, pop, "po")
                        emit_pv(1, 2)
                        emit_xtr(1, 1)
                        emit_outproj(4)
                        emit_pv(1, 3)
                        emit_xtr(1, 2)
                        emit_outproj(5)
                        emit_xtr(1, 3)
                        emit_outproj(6)
                        emit_outproj(7)

            if _DBG:
                nc.sync.dma_start(dbg["dbg_qT"], qT[:, 0, :].bitcast(F32))
                nc.sync.dma_start(dbg["dbg_kT"], kT[:, 0, :].bitcast(F32))
                nc.sync.dma_start(dbg["dbg_v"],
                                  vpa[:, 4].rearrange("p h e -> p (h e)").bitcast(F32))
                nc.sync.dma_start(dbg["dbg_eb"], ebh[0][:].bitcast(F32))
                nc.sync.dma_start(dbg["dbg_pt"], pts[(1, 0)][:].bitcast(F32))
                nc.sync.dma_start(dbg["dbg_xt"], xTs[:, 0, :].bitcast(F32))

    nc.compile()
    return nc


_NC = {}
LAST_RESULT = None


def _get_nc(nreps=1):
    if nreps not in _NC:
        _NC[nreps] = _build_program(nreps)
    return _NC[nreps]


def _prep_inputs(seq, attn_bias, Wq, bq, Wkv, Wout, Wg, bg, mask):
    seq = np.ascontiguousarray(np.asarray(seq, dtype=np.float32))
    attn_bias = np.asarray(attn_bias, dtype=np.float32)
    Wq = np.ascontiguousarray(np.asarray(Wq, dtype=np.float32))
    Wkv = np.ascontiguousarray(np.asarray(Wkv, dtype=np.float32))
    Wout = np.ascontiguousarray(np.asarray(Wout, dtype=np.float32))
    Wg = np.ascontiguousarray(np.asarray(Wg, dtype=np.float32))
    bq = np.ascontiguousarray(np.asarray(bq, dtype=np.float32))
    bg = np.ascontiguousarray(np.asarray(bg, dtype=np.float32))
    b, n, dim = seq.shape
    SC = 4
    CH = n // SC
    in_maps = []
    for c in range(8):
        bi, sc = divmod(c, SC)
        r0 = sc * CH
        kv = np.zeros((NKVROWS, DIM), np.float32)
        lo, hi = r0 - W, r0 + CH + W
        slo, shi = max(lo, 0), min(hi, n)
        kv[slo - lo:shi - lo] = seq[bi, slo:shi]
        seqT = np.ascontiguousarray(kv.T)
        # bias band, transposed and packed per q-half:
        # bandT[half, key_row, OFFS[half][j] + (qi-qlo)*W + q] =
        #   attn_bias[bi, global row of (qi,q), global key row of (j, key_row)]
        bandT = np.full((2, P, 1536), NEG, np.float32)
        for half in (0, 1):
            for j in _HALF_JS[half]:
                qlo, qhi = _q_window_half(j, half)
                kg0 = (sc * NQ + j - 1) * W   # global key row of local key 0
                sk_lo, sk_hi = max(kg0, 0), min(kg0 + W, n)
                if sk_lo >= sk_hi:
                    continue
                for qi in range(qlo, qhi + 1):
                    col0 = _OFFS[half][j] + (qi - qlo) * W
                    g0 = (sc * NQ + qi) * W
                    blk = attn_bias[bi, g0:g0 + W, sk_lo:sk_hi]
                    bandT[half, sk_lo - kg0:sk_hi - kg0, col0:col0 + W] = blk.T
        in_maps.append(dict(seqT=seqT, bandT=bandT, Wq=Wq, Wkv=Wkv, Wg=Wg,
                            Wout=Wout, bq=bq, bg=bg))
    if WIRE_BF16:
        import ml_dtypes
        for im in in_maps:
            for k in ("seqT", "Wq", "Wkv", "Wg", "Wout", "bg", "bandT"):
                im[k] = im[k].astype(ml_dtypes.bfloat16)
    return in_maps


def kernel(seq, attn_bias, Wq, bq, Wkv, Wout, Wg, bg, mask):
    global LAST_RESULT
    nc = _get_nc()
    in_maps = _prep_inputs(seq, attn_bias, Wq, bq, Wkv, Wout, Wg, bg, mask)
    res = run_bass_kernel_spmd(nc, in_maps, core_ids=list(range(8)))
    LAST_RESULT = res
    b, n, dim = np.asarray(seq).shape
    out = np.empty((b, n, dim), np.float32)
    for c in range(8):
        bi, sc = divmod(c, 4)
        out[bi, sc * NQROWS:(sc + 1) * NQROWS] = res.results[c]["out"]
    return out


if __name__ == "__main__":
    rng = np.random.default_rng(0)
    seq = rng.standard_normal((2, 4096, 512), dtype=np.float32)
    bias = rng.standard_normal((2, 4096, 4096), dtype=np.float32) * 0.1
    Wq = rng.standard_normal((512, 512), dtype=np.float32) * 0.02
    Wkv = rng.standard_normal((512, 1024), dtype=np.float32) * 0.02
    Wout = rng.standard_normal((512, 512), dtype=np.float32) * 0.02
    Wg = rng.standard_normal((512, 512), dtype=np.float32) * 0.02
    bq = np.zeros(512, np.float32)
    bg = np.ones(512, np.float32)
    mask = np.ones((2, 4096), bool)
    out = kernel(seq, bias, Wq, bq, Wkv, Wout, Wg, bg, mask)
    print(out.shape, out.dtype)


# revision 73
# speedup vs baseline: 1.5849x; 1.0498x over previous
"""Trainium2 Bass kernel for nn_Attention_87454124081916 (sparse local-window attention).

Reference computation (per batch b, length n=4096, dim=512, 8 heads x 64):
  q = seq @ Wq + bq ; k,v = split(seq @ Wkv) ; g = sigmoid(seq @ Wg + bg)
  local window attention (window=128, keys = prev/cur/next window) with additive
  bias band from attn_bias, softmax over the 384 keys
  out = (attn_out * g) @ Wout

Sharding: 8 cores = 2 batches x 4 sequence chunks of 1024 rows.  Each core gets
its q rows plus a 128-row k/v halo on each side (zero-padded at batch edges) and
the pre-sliced bias band for its rows (out-of-range keys filled with -1e30).
No cross-core communication.  Host prep is layout-only plus a bf16 downcast of
seq/weights/band for the wire (halves the serial input-DMA stream; all matmul
accumulation stays fp32 in PSUM): seq arrives pre-transposed ([dim, rows]) and
the bias band arrives transposed and packed per q-half ([keys, q-slots]), so
the device does no PE transposes of inputs and exps the band in one ACT op per
half.

Device dataflow (per core), everything bf16 on the PE at 1 cyc/row:
  - qT,kT computed in [inner, n] layout; v in natural [n, inner] layout with a
    ones column appended per head (so the softmax denominator Z falls out of
    the PV matmul).
  - S is computed TRANSPOSED per key-tile j: S^T_j = K_j q^T with the
    attention scale folded into the exp's scale operand; exp needs no
    max-subtraction (logits are O(1)).  Both halves' S/exp streams run during
    the projection phase, right behind each head-pair's q/k projections, so
    the ACT exp stream (the second-longest engine stream) starts as early as
    the DMA allows and runs unbroken.
  - P^T = exp(scale*S^T) * exp(bias^T) in bf16; half-1's multiplies run on the
    otherwise idle gpsimd engine.
  - PV runs in NATURAL orientation: O[q,65] = P^T(lhsT) @ V, free dim 65 --
    half the PE columns of the transposed form -- and puts the per-q softmax
    normalizer on the PARTITION axis where it is cheap: rz = 1/Z on [128,8]
    per q-tile via a bit-trick+Newton reciprocal on DVE (keeps the epilogue
    off the ACT table ops), applied via a stride-0-broadcast DVE multiply.
  - gates are computed in natural layout with bg added via a K=1
    outer-product matmul and a one-pass table Sigmoid; the gate block is
    contiguous in the ACT stream so it costs exactly two table loads, and its
    PE matmuls fill the epilogue while exps drain.
  - X = O * g * rz in bf16 is PE-transposed per 128-block into the lhsT of
    the output projection.
  - PSUM is re-scoped mid-program: the S pools close after the last exp so
    the epilogue runs with a double-buffered otz pool and a 3-deep shared
    [P,512] pool for gates / late v-tiles / out-proj.
"""


import os
import sys
from contextlib import ExitStack

import numpy as np

for _p in ("/opt/trn_rl_repo",):
    if _p not in sys.path:
        sys.path.insert(0, _p)

import concourse.bacc as bacc
import concourse.bass as bass
import concourse.hw_specs as hw_specs
import concourse.mybir as mybir
import concourse.tile as tile
from concourse.bass_utils import run_bass_kernel_spmd

F32 = mybir.dt.float32
F32R = mybir.dt.float32r
AF = mybir.ActivationFunctionType

P = 128          # partitions / window size
DIM = 512
INNER = 512
H = 8
D = 64
W = 128
NQ = 8           # q tiles per core
NKV = 10         # kv tiles per core (incl. 1-tile halo each side)
NQROWS = NQ * W      # 1024
NKVROWS = NKV * W    # 1280
NEG = -1.0e30
SCALE = float(D) ** -0.5

# float32r: full-rate fp32 matmul with reduced internal precision.  walrus
# requires matmul operands to be *written* as float32r, so tiles feeding
# matmuls are declared with this dtype and producers cast on write.
MMT = F32R
# bf16 for the attention-probability path (P^T, exp(bias^T), V, X): halves the
# DVE elementwise cost (2x/4x packed modes) at ~0.4% relative error, far
# inside the correctness gate; matmul accumulation stays fp32 in PSUM.
BF16 = mybir.dt.bfloat16

_DBG = os.environ.get("KDBG") == "1"
# ship seq + projection weights to the device in bf16: halves the serial
# input-DMA stream (the projection matmuls then run bf16/bf16; PSUM
# accumulation stays fp32)
WIRE_BF16 = os.environ.get("KF32_WIRE") != "1"


# This kernel's only transcendentals are Exp and Ln.  The ACT table-set picker
# takes the first set containing each function, which puts Exp in
# 'exp_and_others' and Ln in 'natural_log' and reloads the table RAM on every
# alternation (~1.3us each).  Steer both to the combined
# 'natural_log_exp_and_others' set by hiding Exp/Ln from the other sets in the
# table map handed to the placement pass (names and order are kept, so the
# emitted act_func_set_id indices stay aligned with act_info.json and the
# runtime tables genuinely contain the functions used).
_orig_get_activation_tables = hw_specs.get_activation_tables


def _combined_act_tables(arch):
    tabs = dict(_orig_get_activation_tables(arch))
    sig = mybir.ActivationFunctionType.Sigmoid
    out = {}
    for name, funcs in tabs.items():
        # the combined exp/ln set keeps everything; 'sigmoid_and_others'
        # keeps ONLY sigmoid (so Copy/Identity still resolve to the combined
        # set); every other set is emptied.  Result: one sigmoid table trip
        # for the gate block (2 loads), zero other reloads.
        if name == "natural_log_exp_and_others":
            out[name] = funcs
        elif name == "sigmoid_and_others":
            out[name] = {f for f in funcs if f == sig}
        else:
            out[name] = set()
    return out


bacc.get_activation_tables = _combined_act_tables


def _mm(ap):
    # dram-side bitcast for DMA into float32r tiles (bit-identical copy)
    return ap.bitcast(MMT) if ap.dtype == F32 and MMT is not F32 else ap


def _q_window(j):
    """local q-tile window (inclusive) served by local kv tile j."""
    return max(0, j - 2), min(NQ - 1, j)


def _q_window_half(j, half):
    lo, hi = _q_window(j)
    return max(lo, half * 4), min(hi, half * 4 + 3)


_HALF_JS = {0: [0, 1, 2, 3, 4, 5], 1: [4, 5, 6, 7, 8, 9]}

# packed column offsets (j order) shared by the P^T tile and the exp(bias^T)
# tiles; total width is 1536 per half
_OFFS = {}
for _half in (0, 1):
    _OFFS[_half] = {}
    _cum = 0
    for _j in _HALF_JS[_half]:
        _lo, _hi = _q_window_half(_j, _half)
        _OFFS[_half][_j] = _cum
        _cum += (_hi - _lo + 1) * W
    assert _cum == 1536

# S^T PSUM layout: per (half, head) two [P,1024] tiles; each tile holds two
# 384-wide groups at bank offsets 0 and 512 (matmul writes may not cross the
# 512-col PSUM bank boundary).  Each entry is (tile_idx, [(j, dst_off, qlo,
# qhi), ...]): the matmul for j writes q tiles qlo..qhi (padded to >=2 tiles
# so the fp32r free dim is >=256 and the PE runs 1 cyc/row) at tile cols
# dst_off...; emission order guarantees garbage columns from the padding are
# overwritten by later real writes.
# bf16 S operands run 1 cyc/row at any free width, so no fp32r >=256 padding
_S_LAYOUT = {
    0: [(0, [(0, 0, 0, 0), (1, 128, 0, 1), (2, 512, 0, 2)]),
        (1, [(3, 0, 1, 3), (4, 512, 2, 3), (5, 768, 3, 3)])],
    1: [(0, [(4, 0, 4, 4), (5, 128, 4, 5), (6, 512, 4, 6)]),
        (1, [(7, 0, 5, 7), (8, 512, 6, 7), (9, 768, 7, 7)])],
}


def _build_program(nreps=1):
    nc = bacc.Bacc("TRN2", target_bir_lowering=False, debug=False)

    WT = BF16 if WIRE_BF16 else F32
    seqT_d = nc.dram_tensor("seqT", [DIM, NKVROWS], WT, kind="ExternalInput").ap()
    bandT_d = nc.dram_tensor("bandT", [2, P, 1536], WT, kind="ExternalInput").ap()
    Wq_d = nc.dram_tensor("Wq", [DIM, INNER], WT, kind="ExternalInput").ap()
    Wkv_d = nc.dram_tensor("Wkv", [DIM, 2 * INNER], WT, kind="ExternalInput").ap()
    Wg_d = nc.dram_tensor("Wg", [DIM, INNER], WT, kind="ExternalInput").ap()
    Wout_d = nc.dram_tensor("Wout", [INNER, DIM], WT, kind="ExternalInput").ap()
    bq_d = nc.dram_tensor("bq", [INNER], F32, kind="ExternalInput").ap()
    bg_d = nc.dram_tensor("bg", [INNER], WT, kind="ExternalInput").ap()
    out_d = nc.dram_tensor("out", [NQROWS, DIM], F32, kind="ExternalOutput").ap()

    dbg = {}
    if _DBG:
        for nm, shp in [("dbg_qT", [P, NQROWS]), ("dbg_kT", [P, NKVROWS]),
                        ("dbg_v", [P, H * (D + 1)]), ("dbg_eb", [P, 1536]),
                        ("dbg_pt", [P, 1536]), ("dbg_otz", [P, 1024]),
                        ("dbg_rz", [P, 8]), ("dbg_gn", [P, 512]),
                        ("dbg_xt", [P, NQROWS]), ("dbg_seqT", [P, NKVROWS])]:
            dbg[nm] = nc.dram_tensor(nm, shp, F32, kind="ExternalOutput").ap()

    import ml_dtypes
    eye_d = nc.inline_tensor(np.eye(P).astype(ml_dtypes.bfloat16),
                             name="eye").ap()
    ones_d = nc.inline_tensor(
        np.ones((1, P)).astype(ml_dtypes.bfloat16 if WIRE_BF16
                               else np.float32), name="ones").ap()

    with tile.TileContext(nc) as tc:
      for _rep in range(nreps):
        with ExitStack() as ctx:
            # ------------------------------------------------------------------
            # persistent pools + DMA wave
            # ------------------------------------------------------------------
            wpool = ctx.enter_context(tc.tile_pool(name="wpool", bufs=1))
            apool = ctx.enter_context(tc.tile_pool(name="apool", bufs=1))

            WMT = BF16 if WIRE_BF16 else MMT
            seqT = apool.tile([P, 4, NKVROWS], WMT, name="seqT", tag="seqT")
            seq_re = _mm(seqT_d.rearrange("(a p) n -> p a n", p=P))
            bqs = wpool.tile([P, 4], F32, name="bqs", tag="bqs")
            WMTX = BF16 if WIRE_BF16 else MMT
            bgr = wpool.tile([1, INNER], WMTX, name="bgr", tag="bgr")
            ones1 = wpool.tile([1, P], WMTX, name="ones1", tag="ones1")
            # seq chunks stream on the gpsimd software-DGE queue, weights on
            # the SP hardware queue: two parallel input streams instead of
            # one serial 28us stream.
            # every DMA costs a serialized ~625ns HWDGE dispatch slot, so
            # the critical prefix (seq cols + m0 weights) goes absolutely
            # first; tiny late-consumed tensors (bgr/eye/ones) go last
            wq_a = wpool.tile([P, 4, INNER], WMT, name="wq_a", tag="wq_a")
            wk_a = wpool.tile([P, 4, INNER], WMT, name="wk_a", tag="wk_a")
            wq_re = _mm(Wq_d.rearrange("(a p) n -> p a n", p=P))
            wk_re = _mm(Wkv_d[:, 0:INNER].rearrange("(a p) n -> p a n", p=P))
            nc.sync.dma_start(seqT[:, :, 0:256], seq_re[:, :, 0:256])
            nc.sync.dma_start(seqT[:, :, 256:768], seq_re[:, :, 256:768])
            cols = slice(0, P)
            nc.sync.dma_start(wq_a[:, :, cols], wq_re[:, :, cols])
            nc.sync.dma_start(wk_a[:, :, cols], wk_re[:, :, cols])
            nc.sync.dma_start(bqs[:], bq_d.rearrange("(m p) -> p m", p=P))
            nc.sync.dma_start(seqT[:, :, 768:1280], seq_re[:, :, 768:1280])
            bsbs = []
            for half in (0, 1):
                bsb = wpool.tile([P, 1536], BF16 if WIRE_BF16 else F32,
                                 name=f"bsb{half}", tag=f"bsb{half}")
                nc.sync.dma_start(bsb[:], bandT_d[half])
                bsbs.append(bsb)
            for m in range(1, 4):
                cols = slice(m * P, (m + 1) * P)
                nc.sync.dma_start(wq_a[:, :, cols], wq_re[:, :, cols])
                nc.sync.dma_start(wk_a[:, :, cols], wk_re[:, :, cols])
            eye_bf = wpool.tile([P, P], BF16, name="eye_bf_sb", tag="eye_bf")
            nc.sync.dma_start(eye_bf[:], eye_d[:])
            nc.sync.dma_start(bgr[:], _mm(bg_d.rearrange("(a n) -> a n", a=1)))
            nc.sync.dma_start(ones1[:], _mm(ones_d[:]))
            wv_a = wpool.tile([P, 4, INNER], WMT, name="wv_a", tag="wv_a")
            nc.sync.dma_start(
                wv_a[:],
                _mm(Wkv_d[:, INNER:2 * INNER].rearrange("(a p) n -> p a n", p=P)))
            wg_a = wpool.tile([P, 4, INNER], WMT, name="wg_a", tag="wg_a")
            nc.sync.dma_start(wg_a[:], _mm(Wg_d.rearrange("(a p) n -> p a n", p=P)))
            wo_a = wpool.tile([P, 4, DIM], WMT, name="wo_a", tag="wo_a")
            nc.sync.dma_start(
                wo_a[:], _mm(Wout_d.rearrange("(a p) n -> p a n", p=P)))

            # ---- persistent activations --------------------------------------
            # transposed activations [128, ktile, n]; head h lives at partition
            # rows (h%2)*64 of ktile h//2
            qT = apool.tile([P, 4, NQROWS], BF16, name="qT", tag="qT")
            kT = apool.tile([P, 4, NKVROWS], BF16, name="kT", tag="kT")
            gn = apool.tile([P, NQ, INNER], BF16, name="gn", tag="gn")
            xTs = apool.tile([P, 4, NQROWS], BF16, name="xTs", tag="xTs")
            # v natural, padded per head with a ones column: [128, 10, 8, 65]
            vpa = apool.tile([P, NKV, H, D + 1], BF16, name="vpa", tag="vpa")
            # exp(bias^T) packed per half: [128 keys, 1536]
            ebh = [apool.tile([P, 1536], BF16, name=f"ebh{i}", tag=f"ebh{i}")
                   for i in (0, 1)]
            pts = {}   # (half, h) -> pt tile
            stp_cm = tc.tile_pool(name="stp", bufs=2, space="PSUM")
            stp = stp_cm.__enter__()

            def emit_s_head(half, h, ptp):
                """S^T matmuls + exp + exp(bias) multiply for one head."""
                m, r0 = h // 2, (h % 2) * D
                pt = ptp.tile([P, 1536], BF16, name="pt", tag="pt")
                pts[(half, h)] = pt
                for ti, writes in _S_LAYOUT[half]:
                    st = stp.tile([P, 1024], F32, name="st", tag="st")
                    for (j, dst, qlo, qhi) in writes:
                        nc.tensor.matmul(
                            st[:, dst:dst + (qhi - qlo + 1) * W],
                            kT[r0:r0 + D, m, j * W:(j + 1) * W],
                            qT[r0:r0 + D, m, qlo * W:(qhi + 1) * W],
                            start=True, stop=True,
                        )
                    nc.scalar.activation(
                        pt[:, ti * 768:(ti + 1) * 768].rearrange(
                            "p (a c) -> p a c", c=384),
                        st[:].rearrange("p (a c) -> p a c", c=512)[:, :, 0:384],
                        AF.Exp, scale=SCALE)
                # half-1 multiplies on the (otherwise idle) Pool engine:
                # slow but off the DVE spine, done before PV(1,*) needs them
                eng = nc.gpsimd if half == 1 else nc.vector
                eng.tensor_mul(pt[:], pt[:], ebh[half][:])

            # ------------------------------------------------------------------
            # phase 1: projections (+ gates on ACT's idle window) with the
            # half-0 S/exp stream interleaved per head-pair m
            # ------------------------------------------------------------------
            with tc.tile_pool(name="ptp", bufs=16) as ptp:
              with tc.tile_pool(name="bsbp", bufs=2) as bsbp, \
                   tc.tile_pool(name="wsp", bufs=1) as wsp, \
                   tc.tile_pool(name="pjps", bufs=4, space="PSUM") as pjps:

                def emit_gproj(t, pool, tag):
                    # natural-layout gate projection; bg added via a K=1
                    # outer-product matmul; one-pass sigmoid (own table set)
                    pg = pool.tile([P, 512], F32, name="pg", tag=tag)
                    nc.tensor.matmul(pg[:], ones1[:], bgr[:],
                                     start=True, stop=False)
                    for kk in range(4):
                        nc.tensor.matmul(
                            pg[:],
                            seqT[:, kk, (t + 1) * P:(t + 2) * P],
                            wg_a[:, kk, :],
                            start=False, stop=(kk == 3),
                        )
                    nc.scalar.activation(gn[:, t, :], pg[:], AF.Sigmoid)

                def emit_vproj(nt, pool, tag, eng=None):
                    pv_ = pool.tile([P, 512], F32, name="pv_", tag=tag)
                    for kk in range(4):
                        nc.tensor.matmul(
                            pv_[:],
                            seqT[:, kk, nt * P:(nt + 1) * P],
                            wv_a[:, kk, :],
                            start=(kk == 0), stop=(kk == 3),
                        )
                    if eng == "act":
                        nc.scalar.activation(
                            vpa[:, nt, :, 0:D],
                            pv_[:].rearrange("p (h e) -> p h e", e=D),
                            AF.Copy)
                    else:
                        nc.vector.tensor_copy(
                            vpa[:, nt, :, 0:D],
                            pv_[:].rearrange("p (h e) -> p h e", e=D),
                        )

                # band exp (band DMA'd early on SP)
                for half in (0, 1):
                    nc.scalar.activation(ebh[half][:], bsbs[half][:], AF.Exp)

                # q / k projections per head-pair m with BOTH halves' S/exp
                # streams right behind: the ACT exp stream starts as early
                # as possible and runs unbroken; v/g projections fill the PE
                # later, under the exps
                def emit_qproj(m, s2):
                    cols = slice(W + s2 * 512, W + (s2 + 1) * 512)
                    pq = pjps.tile([P, 512], F32, name="pq", tag="pj")
                    for kk in range(4):
                        nc.tensor.matmul(
                            pq[:],
                            wq_a[:, kk, m * P:(m + 1) * P],
                            seqT[:, kk, cols],
                            start=(kk == 0), stop=(kk == 3),
                        )
                    nc.vector.tensor_scalar_add(
                        qT[:, m, s2 * 512:(s2 + 1) * 512], pq[:],
                        bqs[:, m:m + 1])

                def emit_kproj(m, s3):
                    wdt = 512 if s3 < 2 else 256
                    cols = slice(s3 * 512, s3 * 512 + wdt)
                    pk = pjps.tile([P, 512], F32, name="pk", tag="pj")
                    for kk in range(4):
                        nc.tensor.matmul(
                            pk[:, 0:wdt],
                            wk_a[:, kk, m * P:(m + 1) * P],
                            seqT[:, kk, cols],
                            start=(kk == 0), stop=(kk == 3),
                        )
                    nc.vector.tensor_copy(kT[:, m, cols], pk[:, 0:wdt])

                for m in range(4):
                    # half-0's S needs only q-s2=0 and k-s3=0,1: emit those
                    # first so the exp stream starts as early as possible
                    emit_qproj(m, 0)
                    emit_kproj(m, 0)
                    emit_kproj(m, 1)
                    emit_s_head(0, 2 * m, ptp)
                    emit_s_head(0, 2 * m + 1, ptp)
                    emit_qproj(m, 1)
                    emit_kproj(m, 2)
                    emit_s_head(1, 2 * m, ptp)
                    emit_s_head(1, 2 * m + 1, ptp)
                for nt in range(6):
                    emit_vproj(nt, pjps, "pj")

                if _DBG:
                    nc.sync.dma_start(dbg["dbg_seqT"], seqT[:, 0, :].bitcast(F32))



              # ----------------------------------------------------------------
              # phase 2: attention epilogue per q tile + half-1 S stream
              # (pjps closed: its PSUM banks are recycled for the epilogue)
              # ----------------------------------------------------------------
              if True:
                with tc.tile_pool(name="xgp", bufs=3) as xgp, \
                     tc.tile_pool(name="zrp", bufs=3) as zrp, \
                     tc.tile_pool(name="osb", bufs=3) as osb:

                    qstate = {}
                    pp = {}

                    def emit_pv(half, qi):
                        """PV matmuls + Z chain + gate multiply for one q
                        tile (the rz multiply is deferred to emit_xtr so the
                        ACT Ln/Exp latency stays off the DVE stream)."""
                        gq = half * 4 + qi
                        otz = pp["otz"].tile([P, 1024], F32, name="otz",
                                             tag="otz")
                        for h in range(H):
                            js = [j for j in _HALF_JS[half]
                                  if _q_window_half(j, half)[0] <= gq
                                  <= _q_window_half(j, half)[1]]
                            dst0 = (h // 4) * 512 + (h % 4) * (D + 1)
                            for ji, j in enumerate(js):
                                lo = _q_window_half(j, half)[0]
                                off = _OFFS[half][j] + (gq - lo) * W
                                nc.tensor.matmul(
                                    otz[:, dst0:dst0 + D + 1],
                                    pts[(half, h)][:, off:off + W],
                                    vpa[:, j, h, :],
                                    start=(ji == 0), stop=(ji == len(js) - 1),
                                )
                        # rz = 1/Z per (q row, head) [128,8] via a Newton
                        # step on DVE (bit-trick seed): keeps the epilogue
                        # entirely off the ACT table ops so the gate
                        # sigmoids stay contiguous (no table reloads)
                        zv = otz[:].rearrange("p (g s) -> p g s", g=2)[
                            :, :, 0:4 * (D + 1)].rearrange(
                            "p g (h e) -> p g h e", e=D + 1)
                        zcol = zv[:, :, :, D:D + 1]
                        I32 = mybir.dt.int32
                        zl = zrp.tile([P, 8], F32, name="zl", tag="zl")
                        nc.vector.tensor_scalar(
                            zl[:].bitcast(I32).rearrange(
                                "p (g h e) -> p g h e", g=2, e=1),
                            zcol.bitcast(I32), -1, 0x7EF311C3,
                            mybir.AluOpType.mult, mybir.AluOpType.add)
                        tt = zrp.tile([P, 8], F32, name="tt", tag="tt")
                        nc.vector.tensor_mul(
                            tt[:].rearrange("p (g h e) -> p g h e", g=2, e=1),
                            zcol, zl[:].rearrange("p (g h e) -> p g h e",
                                                  g=2, e=1))
                        nc.vector.tensor_scalar(
                            tt[:], tt[:], -1.0, 2.0,
                            mybir.AluOpType.mult, mybir.AluOpType.add)
                        rz = zrp.tile([P, 8], F32, name="rz", tag="rz")
                        nc.vector.tensor_mul(rz[:], zl[:], tt[:])
                        # gate multiply first -- it does not depend on rz
                        xg = xgp.tile([P, 512], BF16, name="xg", tag="xg")
                        nc.vector.tensor_mul(
                            xg[:].rearrange("p (g h e) -> p g h e", g=2, e=D),
                            zv[:, :, :, 0:D],
                            gn[:, gq, :].rearrange("p (g h e) -> p g h e",
                                                   g=2, e=D))
                        qstate[gq] = (xg, rz)
                        if _DBG and gq == 0:
                            nc.sync.dma_start(dbg["dbg_otz"], otz[:])
                            nc.sync.dma_start(dbg["dbg_rz"], rz[:])
                            nc.sync.dma_start(dbg["dbg_gn"], gn[:, 0, :])

                    def emit_xtr(half, qi):
                        """rz multiply + X transpose + xTs copy."""
                        gq = half * 4 + qi
                        xg, rz = qstate.pop(gq)
                        xn = xgp.tile([P, 512], BF16, name="xn", tag="xn")
                        # half-1: DVE is the tail bottleneck; Pool is idle
                        # there, so shift the rz multiply off DVE
                        veng = nc.gpsimd if half == 1 else nc.vector
                        veng.tensor_mul(
                            xn[:].rearrange("p (g h e) -> p g h e", g=2, e=D),
                            xg[:].rearrange("p (g h e) -> p g h e", g=2, e=D),
                            rz[:].rearrange("p (g h) -> p g h", g=2)
                                 .unsqueeze(3).broadcast_to([P, 2, 4, D]))
                        # transpose X (bf16, 1 cyc/row) into xTs
                        xt = pp["xt"].tile([P, 4, P], BF16, name="xt",
                                           tag="xt")
                        for b in range(4):
                            nc.tensor.transpose(
                                xt[:, b, :], xn[:, b * P:(b + 1) * P],
                                eye_bf[:])
                        nc.vector.tensor_copy(
                            xTs[:, :, gq * P:(gq + 1) * P], xt[:])

                    def emit_outproj(t):
                        po = pp["po"].tile([P, DIM], F32, name="po", tag="po")
                        ot = osb.tile([P, DIM], F32, name="ot", tag="ot")
                        for mm in range(4):
                            nc.tensor.matmul(
                                po[:],
                                xTs[:, mm, t * P:(t + 1) * P],
                                wo_a[:, mm, :],
                                start=(mm == 0), stop=(mm == 3),
                            )
                        if t >= 4:
                            nc.scalar.activation(ot[:], po[:], AF.Copy)
                        else:
                            nc.vector.tensor_copy(ot[:], po[:])
                        nc.sync.dma_start(out_d[t * P:(t + 1) * P, :], ot[:])

                    # ---- epilogues: S/exp all done in phase 1; stp is
                    # closed so the epilogue runs with double-buffered otz
                    # and a deep shared [P,512] pool for gates / late
                    # v-tiles / out-proj: otz(4) + xt(1) + po(3) = 8 banks.
                    stp_cm.__exit__(None, None, None)
                    with tc.tile_pool(name="otzp", bufs=2,
                                      space="PSUM") as otzp, \
                         tc.tile_pool(name="xtp", bufs=1,
                                      space="PSUM") as xtp, \
                         tc.tile_pool(name="pop", bufs=3,
                                      space="PSUM") as pop:
                        pp["otz"], pp["xt"], pp["po"] = otzp, xtp, pop
                        # ones column via ACT Copy(0*x+1) -- walrus rejects
                        # non-zero memsets for 2-byte dtypes
                        nc.scalar.activation(
                            vpa[:, :, :, D:D + 1],
                            eye_bf[:, 0:NKV * H].rearrange(
                                "p (a b c) -> p a b c", b=H, c=1),
                            AF.Copy, bias=1.0, scale=0.0)
                        emit_gproj(0, pop, "po")
                        emit_gproj(1, pop, "po")
                        emit_pv(0, 0)
                        emit_gproj(2, pop, "po")
                        emit_pv(0, 1)
                        emit_xtr(0, 0)
                        emit_gproj(3, pop, "po")
                        emit_pv(0, 2)
                        emit_xtr(0, 1)
                        emit_outproj(0)
                        emit_gproj(4, pop, "po")
                        emit_pv(0, 3)
                        emit_xtr(0, 2)
                        emit_outproj(1)
                        emit_vproj(6, pop, "po")
                        emit_gproj(5, pop, "po")
                        emit_vproj(7, pop, "po")
                        emit_pv(1, 0)
                        emit_xtr(0, 3)
                        emit_outproj(2)
                        emit_gproj(6, pop, "po")
                        emit_vproj(8, pop, "po")
                        emit_pv(1, 1)
                        emit_xtr(1, 0)
                        emit_outproj(3)
                        emit_gproj(7, pop, "po")
                        emit_vproj(9, pop, "po")
                        emit_pv(1, 2)
                        emit_xtr(1, 1)
                        emit_outproj(4)
                        emit_pv(1, 3)
                        emit_xtr(1, 2)
                        emit_outproj(5)
                        emit_xtr(1, 3)
                        emit_outproj(6)
                        emit_outproj(7)

            if _DBG:
                nc.sync.dma_start(dbg["dbg_qT"], qT[:, 0, :].bitcast(F32))
                nc.sync.dma_start(dbg["dbg_kT"], kT[:, 0, :].bitcast(F32))
                nc.sync.dma_start(dbg["dbg_v"],
                                  vpa[:, 4].rearrange("p h e -> p (h e)").bitcast(F32))
                nc.sync.dma_start(dbg["dbg_eb"], ebh[0][:].bitcast(F32))
                nc.sync.dma_start(dbg["dbg_pt"], pts[(1, 0)][:].bitcast(F32))
                nc.sync.dma_start(dbg["dbg_xt"], xTs[:, 0, :].bitcast(F32))

    nc.compile()
    return nc


_NC = {}
LAST_RESULT = None


def _get_nc(nreps=1):
    if nreps not in _NC:
        _NC[nreps] = _build_program(nreps)
    return _NC[nreps]


def _prep_inputs(seq, attn_bias, Wq, bq, Wkv, Wout, Wg, bg, mask):
    seq = np.ascontiguousarray(np.asarray(seq, dtype=np.float32))
    attn_bias = np.asarray(attn_bias, dtype=np.float32)
    Wq = np.ascontiguousarray(np.asarray(Wq, dtype=np.float32))
    Wkv = np.ascontiguousarray(np.asarray(Wkv, dtype=np.float32))
    Wout = np.ascontiguousarray(np.asarray(Wout, dtype=np.float32))
    Wg = np.ascontiguousarray(np.asarray(Wg, dtype=np.float32))
    bq = np.ascontiguousarray(np.asarray(bq, dtype=np.float32))
    bg = np.ascontiguousarray(np.asarray(bg, dtype=np.float32))
    b, n, dim = seq.shape
    SC = 4
    CH = n // SC
    in_maps = []
    for c in range(8):
        bi, sc = divmod(c, SC)
        r0 = sc * CH
        kv = np.zeros((NKVROWS, DIM), np.float32)
        lo, hi = r0 - W, r0 + CH + W
        slo, shi = max(lo, 0), min(hi, n)
        kv[slo - lo:shi - lo] = seq[bi, slo:shi]
        seqT = np.ascontiguousarray(kv.T)
        # bias band, transposed and packed per q-half:
        # bandT[half, key_row, OFFS[half][j] + (qi-qlo)*W + q] =
        #   attn_bias[bi, global row of (qi,q), global key row of (j, key_row)]
        bandT = np.full((2, P, 1536), NEG, np.float32)
        for half in (0, 1):
            for j in _HALF_JS[half]:
                qlo, qhi = _q_window_half(j, half)
                kg0 = (sc * NQ + j - 1) * W   # global key row of local key 0
                sk_lo, sk_hi = max(kg0, 0), min(kg0 + W, n)
                if sk_lo >= sk_hi:
                    continue
                for qi in range(qlo, qhi + 1):
                    col0 = _OFFS[half][j] + (qi - qlo) * W
                    g0 = (sc * NQ + qi) * W
                    blk = attn_bias[bi, g0:g0 + W, sk_lo:sk_hi]
                    bandT[half, sk_lo - kg0:sk_hi - kg0, col0:col0 + W] = blk.T
        in_maps.append(dict(seqT=seqT, bandT=bandT, Wq=Wq, Wkv=Wkv, Wg=Wg,
                            Wout=Wout, bq=bq, bg=bg))
    if WIRE_BF16:
        import ml_dtypes
        for im in in_maps:
            for k in ("seqT", "Wq", "Wkv", "Wg", "Wout", "bg", "bandT"):
                im[k] = im[k].astype(ml_dtypes.bfloat16)
    return in_maps


def kernel(seq, attn_bias, Wq, bq, Wkv, Wout, Wg, bg, mask):
    global LAST_RESULT
    nc = _get_nc()
    in_maps = _prep_inputs(seq, attn_bias, Wq, bq, Wkv, Wout, Wg, bg, mask)
    res = run_bass_kernel_spmd(nc, in_maps, core_ids=list(range(8)))
    LAST_RESULT = res
    b, n, dim = np.asarray(seq).shape
    out = np.empty((b, n, dim), np.float32)
    for c in range(8):
        bi, sc = divmod(c, 4)
        out[bi, sc * NQROWS:(sc + 1) * NQROWS] = res.results[c]["out"]
    return out


if __name__ == "__main__":
    rng = np.random.default_rng(0)
    seq = rng.standard_normal((2, 4096, 512), dtype=np.float32)
    bias = rng.standard_normal((2, 4096, 4096), dtype=np.float32) * 0.1
    Wq = rng.standard_normal((512, 512), dtype=np.float32) * 0.02
    Wkv = rng.standard_normal((512, 1024), dtype=np.float32) * 0.02
    Wout = rng.standard_normal((512, 512), dtype=np.float32) * 0.02
    Wg = rng.standard_normal((512, 512), dtype=np.float32) * 0.02
    bq = np.zeros(512, np.float32)
    bg = np.ones(512, np.float32)
    mask = np.ones((2, 4096), bool)
    out = kernel(seq, bias, Wq, bq, Wkv, Wout, Wg, bg, mask)
    print(out.shape, out.dtype)
